# revision 1
# baseline (speedup 1.0000x reference)
"""ClsbdCRF message-passing kernel for 8 Trainium2 NeuronCores.

Sharding: core i handles batch b = i//2 and image-row half i%2 (64 output
rows each, with span-2 halos sliced host-side).  Per-core SBUF layout puts
W=128 on partitions and (C, H) on the free dimension, so the 5x5 stencil
becomes partition-offset (dy) + free-offset (dx) access patterns.

Math per core (fp32):
  pl   = 1 - ent/ln(C),  ent = -sum_c x ln(x+eps)
  xp   = x * pl
  g1_d = exp(-0.5 * ||f(x) - f(x+d)||^2)            (12 taps + mirrors + center)
  g2_t = ring-max propagation of unfolded clsbd map  (24 taps + center=0)
  w_neg_t = 2*g1_t - ln(g2_t+eps)          (x5 at the end)
  w_pos_t = ln(1 - g2_t + eps)             (x-5 at the end)
  msg[c,h,w] = sum_t w_t[h,w] * xp[c, (h,w)+t]

Boundary handling: H is zero-padded host-side (feats big-padded so the
pairwise gaussian underflows to exactly 0 out of image); W taps use
restricted partition ranges with pre-zeroed destination tiles.
"""

import math

import numpy as np

B, C, H, W, D = 4, 21, 128, 128, 5
SPAN = 2
EPS = 1e-5
HP = 64          # output rows per core
HE = HP + 4      # input / clsbd row extent (halo 2 each side)
FE = HP + 8      # feats row extent (halo 4 each side)
BIGPAD = 1000.0  # feats pad value; (BIGPAD-x)^2 makes exp() underflow to 0
COMPAT_PAIR = 10.0
COMPAT_CLSBD = 5.0

RING1 = [(-1, -1), (-1, 0), (-1, 1), (0, -1), (0, 1), (1, -1), (1, 0), (1, 1)]
RING2 = [(-2, -2), (-2, -1), (-2, 0), (-2, 1), (-2, 2), (-1, -2), (-1, 2),
         (0, -2), (0, 2), (1, -2), (1, 2), (2, -2), (2, -1), (2, 0), (2, 1),
         (2, 2)]
EXP1 = [0, 0, 1, 2, 2, 0, 2, 3, 4, 5, 7, 5, 5, 6, 7, 7]
EXP2 = [0, 1, 1, 1, 2, 3, 4, 3, 4, 3, 4, 5, 6, 6, 6, 7]
# taps whose pairwise gaussian is computed directly; mirrors are shifted reads
DIRTAPS = [(dx, dy) for dx in range(-SPAN, SPAN + 1)
           for dy in range(-SPAN, SPAN + 1) if (dx, dy) > (0, 0)]
ALLTAPS = [(dx, dy) for dx in range(-SPAN, SPAN + 1)
           for dy in range(-SPAN, SPAN + 1)]

GP_NTAPS = 8
_cache = {}


def _wrange(dy):
    return max(0, -dy), W - max(0, dy)


def _build():
    import concourse.bacc as bacc
    import concourse.mybir as mybir
    from concourse.tile import TileContext

    f32 = mybir.dt.float32
    Act = mybir.ActivationFunctionType
    Alu = mybir.AluOpType

    nc = bacc.Bacc()
    x_d = nc.declare_dram_parameter("x", [W, C, HE], f32, isOutput=False)
    f_d = nc.declare_dram_parameter("f", [W, D, FE], f32, isOutput=False)
    s_d = nc.declare_dram_parameter("s", [W, HE], f32, isOutput=False)
    o_d = nc.declare_dram_parameter("out", [2, W, C, HP], f32, isOutput=True)

    DYS = [-2, -1, 1, 2]

    # Pre-TileContext constants, covered by an all-engine barrier (same
    # pattern Bass.__init__ uses) so consumers never need a sync wait.
    def _const_sbuf(name, shape, val):
        t = nc.alloc_sbuf_tensor(name, shape, f32)
        nc.gpsimd.memset(t.ap(), val)
        return t.ap()

    zt = _const_sbuf("zt_const", [W, C, HE], 0.0)
    bpad = _const_sbuf("bpad_const", [W, D, FE], BIGPAD)
    b_eps = _const_sbuf("b_eps", [W, 1], EPS)
    b_ln2 = _const_sbuf("b_ln2", [W, 1], math.log(2.0))
    b_1eps = _const_sbuf("b_1eps", [W, 1], 1.0 + EPS)
    nc.const_aps.aps[(f32, EPS)] = b_eps

    # partition-shift matrices: S_dy[k, m] = 1 iff k = m + dy, so
    # (S_dy^T @ x)[m] = x[m+dy] with zero rows outside [0, W) — PE does
    # the partition shift straight into PSUM, no DMA descriptor storms.
    s_mat = {}
    for dy in (-2, -1, 1, 2):
        t = nc.alloc_sbuf_tensor(f"shift_{dy}", [W, W], f32)
        nc.gpsimd.memset(t.ap(), 0.0)
        nc.gpsimd.affine_select(
            out=t.ap(), in_=t.ap(), compare_op=mybir.AluOpType.not_equal,
            fill=1.0, base=-dy, pattern=[[-1, W]], channel_multiplier=1)
        s_mat[dy] = t.ap()
    nc.all_engine_barrier()

    # taps owned end-to-end by GpSimd (ready earliest: dy=0 needs no
    # shifted xp at all). DVE keeps the rest.
    GP_TAPS = [(1, 0), (2, 0), (-1, 0), (-2, 0)][:GP_NTAPS]

    with TileContext(nc) as tc:
        with (
            tc.tile_pool(name="io", bufs=1) as io,
            tc.tile_pool(name="g1p", bufs=1) as g1p,
            tc.tile_pool(name="g2p", bufs=1) as g2p,
            tc.tile_pool(name="wp", bufs=1) as wp,
            tc.tile_pool(name="lp", bufs=3) as lp,
            tc.tile_pool(name="mp", bufs=2) as mp,
            tc.tile_pool(name="sc", bufs=1) as sc,
            tc.tile_pool(name="scr", bufs=2) as scr,
            tc.tile_pool(name="dr", bufs=1, space="DRAM") as dr,
            tc.tile_pool(name="psp", bufs=2, space="PSUM") as psp,
        ):
            # ---- phase 0: all DRAM loads up front ----
            x_t = io.tile([W, C, HE], f32, tag="x")
            f_t = io.tile([W, D, FE], f32, tag="f")
            s_t = io.tile([W, HE], f32, tag="s")
            nc.sync.dma_start(out=x_t[:], in_=x_d[:])
            nc.sync.dma_start(out=f_t[:], in_=f_d[:])
            nc.sync.dma_start(out=s_t[:], in_=s_d[:])

            def _shift_load(pool, tag, shape, dram, padsrc, dy, eng=None):
                # dy-shifted copy straight from DRAM (one descriptor),
                # out-of-range partitions filled from a barrier-covered const
                eng = eng or nc.sync
                t = pool.tile(shape, f32, tag=tag)
                a, b = _wrange(dy)
                if a > 0:
                    eng.dma_start(out=t[:a], in_=padsrc[:a])
                if b < W:
                    eng.dma_start(out=t[b:], in_=padsrc[b:])
                eng.dma_start(out=t[a:b], in_=dram[a + dy:b + dy])
                return t

            f_s, s_s = {0: f_t}, {0: s_t}
            for dy in DYS:
                f_s[dy] = _shift_load(io, f"fs_{dy}", [W, D, FE], f_d, bpad,
                                      dy)
                s_s[dy] = _shift_load(io, f"ss_{dy}", [W, HE], s_d,
                                      zt[:, 0, :HE], dy)

            # ---- polarness ----
            lnx = sc.tile([W, C, HE], f32, tag="lnx")
            nc.scalar.activation(lnx[:], x_t[:], Act.Ln, bias=b_eps[:], scale=1.0)
            xl = sc.tile([W, C, HE], f32, tag="xl")
            nc.vector.tensor_mul(xl[:], x_t[:], lnx[:])
            ent = sc.tile([W, HE], f32, tag="ent")
            nc.vector.tensor_reduce(
                out=ent[:], in_=xl[:].rearrange("p c h -> p h c"),
                axis=mybir.AxisListType.X, op=Alu.add)
            pl = sc.tile([W, HE], f32, tag="pl")
            # ent holds sum_c x*ln(x+eps) = -entropy
            nc.scalar.activation(pl[:], ent[:], Act.Copy,
                                 bias=1.0, scale=1.0 / math.log(C))
            xp = io.tile([W, C, HE], f32, tag="xp")
            nc.vector.tensor_mul(
                xp[:], x_t[:], pl[:, None, :].broadcast_to((W, C, HE)))
            # xp dy-shifts: 3 matmuls per dy (<=512 fp32 moving-operand
            # cap; 512-f32 slices stay single-bank). 2 PSUM slots rotate
            # through the dy groups, so products must consume dy-major.
            xp_flat = xp[:].rearrange("p c h -> p (c h)")
            FSL = [(0, 512), (512, 1024), (1024, C * HE)]
            xp_s = {0: xp}
            for dy in (-1, 1, -2, 2):
                t = psp.tile([W, C, HE], f32, tag="xps")
                tf = t[:].rearrange("p c h -> p (c h)")
                for (n0, n1) in FSL:
                    nc.tensor.matmul(tf[:, n0:n1], s_mat[dy],
                                     xp_flat[:, n0:n1], start=True, stop=True)
                xp_s[dy] = t

            # ---- pairwise gaussian (12 direct taps; value stored = 2*g1) ----
            # BIGPAD-shifted feats make out-of-image taps underflow to 0.
            g1t = {}
            g1d = {}
            for (dx, dy) in DIRTAPS:
                g1 = g1p.tile([W, HE], f32, tag=f"g1_{dx}_{dy}")
                diff = scr.tile([W, D, HE], f32, tag="diff")
                nc.vector.tensor_sub(
                    diff[:], f_t[:, :, 2:2 + HE],
                    f_s[dy][:, :, 2 + dx:2 + dx + HE])
                sq = scr.tile([W, D, HE], f32, tag="sq")
                nc.scalar.square(sq[:], diff[:])
                ssum = scr.tile([W, HE], f32, tag="ssum")
                nc.vector.tensor_reduce(
                    out=ssum[:], in_=sq[:].rearrange("p d h -> p h d"),
                    axis=mybir.AxisListType.X, op=Alu.add)
                nc.scalar.activation(g1[:], ssum[:], Act.Exp,
                                     bias=b_ln2[:], scale=-0.5)
                g1t[(dx, dy)] = g1
                if dy != 0:
                    gd = dr.tile([W, HE], f32, tag=f"g1d_{dx}_{dy}")
                    nc.scalar.dma_start(out=gd[:], in_=g1[:])
                    g1d[(dx, dy)] = gd
            # dy-shifted mirror copies via DRAM roundtrip
            g1s = {}
            for (dx, dy) in DIRTAPS:
                if dy == 0:
                    g1s[(dx, dy)] = g1t[(dx, dy)]
                    continue
                a, b = _wrange(-dy)
                t = g1p.tile([W, HE], f32, tag=f"g1s_{dx}_{dy}")
                if a > 0:
                    nc.scalar.dma_start(out=t[:a], in_=zt[:a, 0, :HE])
                if b < W:
                    nc.scalar.dma_start(out=t[b:], in_=zt[b:, 0, :HE])
                nc.scalar.dma_start(out=t[a:b],
                                    in_=g1d[(dx, dy)][a - dy:b - dy])
                g1s[(dx, dy)] = t

            # ---- clsbd gaussian: ring max propagation ----
            tmp1 = [s_s[dy][:, 2 + dx:2 + dx + HP] for (dx, dy) in RING1]
            g2t = {t: tmp1[j] for j, t in enumerate(RING1)}
            for k, (dx, dy) in enumerate(RING2):
                t2 = g2p.tile([W, HP], f32, tag=f"t2_{k}")
                nc.vector.tensor_max(t2[:], tmp1[EXP1[k]], tmp1[EXP2[k]])
                nc.vector.tensor_max(
                    t2[:], t2[:], s_s[dy][:, 2 + dx:2 + dx + HP])
                g2t[(dx, dy)] = t2

            # ---- weights for all 24 taps ----
            NT = [t for t in ALLTAPS if t != (0, 0)]
            # GpSimd-owned taps first in emission so its chain starts early
            DYRANK = {0: -1, -1: 0, 1: 1, -2: 2, 2: 3}
            NT.sort(key=lambda t: (t not in GP_TAPS, DYRANK[t[1]]))
            wns, lnps = {}, {}
            for (dx, dy) in NT:
                g2 = g2t[(dx, dy)]
                g2ap = g2[:] if hasattr(g2, "tag") else g2
                lnn = lp.tile([W, HP], f32, tag="lnn")
                nc.scalar.activation(lnn[:], g2ap, Act.Ln, bias=b_eps[:],
                                     scale=1.0)
                lnp = wp.tile([W, HP], f32, tag=f"lnp_{dx}_{dy}")
                nc.scalar.activation(lnp[:], g2ap, Act.Ln,
                                     bias=b_1eps[:], scale=-1.0)
                wn = wp.tile([W, HP], f32, tag=f"wn_{dx}_{dy}")
                if (dx, dy) > (0, 0):
                    g1ap = g1t[(dx, dy)][:, 2:2 + HP]
                else:
                    g1ap = g1s[(-dx, -dy)][:, 2 + dx:2 + dx + HP]
                nc.vector.tensor_sub(wn[:], g1ap, lnn[:])
                wns[(dx, dy)] = wn
                lnps[(dx, dy)] = lnp

            # ---- products + accumulation ----
            accn = io.tile([W, C, HP], f32, tag="accn")
            accp = io.tile([W, C, HP], f32, tag="accp")
            accn2 = io.tile([W, C, HP], f32, tag="accn2")
            accp2 = io.tile([W, C, HP], f32, tag="accp2")
            xp_c = xp[:, :, 2:2 + HP]
            nc.vector.tensor_scalar_mul(accn[:], xp_c, 2.0 - math.log(EPS))
            nc.vector.tensor_scalar_mul(accp[:], xp_c, math.log(1.0 + EPS))
            gp_first = [True]
            for (dx, dy) in NT:
                wnb = wns[(dx, dy)][:, None, :].broadcast_to((W, C, HP))
                lpb = lnps[(dx, dy)][:, None, :].broadcast_to((W, C, HP))
                xpap = xp_s[dy][:, :, 2 + dx:2 + dx + HP]
                if (dx, dy) in GP_TAPS:
                    if gp_first[0]:
                        nc.gpsimd.tensor_mul(accn2[:], wnb, xpap)
                        nc.gpsimd.tensor_mul(accp2[:], lpb, xpap)
                        gp_first[0] = False
                    else:
                        tn = mp.tile([W, C, HP], f32, tag="tng")
                        nc.gpsimd.tensor_mul(tn[:], wnb, xpap)
                        nc.gpsimd.tensor_add(accn2[:], accn2[:], tn[:])
                        tp = mp.tile([W, C, HP], f32, tag="tpg")
                        nc.gpsimd.tensor_mul(tp[:], lpb, xpap)
                        nc.gpsimd.tensor_add(accp2[:], accp2[:], tp[:])
                else:
                    tn = mp.tile([W, C, HP], f32, tag="tn")
                    nc.vector.tensor_mul(tn[:], wnb, xpap)
                    nc.vector.tensor_add(accn[:], accn[:], tn[:])
                    tp = mp.tile([W, C, HP], f32, tag="tp")
                    nc.vector.tensor_mul(tp[:], lpb, xpap)
                    nc.vector.tensor_add(accp[:], accp[:], tp[:])

            nc.vector.tensor_add(accn[:], accn[:], accn2[:])
            nc.vector.tensor_add(accp[:], accp[:], accp2[:])
            nc.scalar.activation(accn[:], accn[:], Act.Copy,
                                 bias=0.0, scale=COMPAT_CLSBD)
            nc.scalar.activation(accp[:], accp[:], Act.Copy,
                                 bias=0.0, scale=-COMPAT_CLSBD)
            nc.sync.dma_start(out=o_d[0], in_=accn[:])
            nc.sync.dma_start(out=o_d[1], in_=accp[:])
    nc.finalize()
    return nc


_last_results = None


def kernel(input, feats, clsbd_feats, label=None, **_ignored):
    global _last_results
    from concourse.bass_utils import run_bass_kernel_spmd

    x = np.asarray(input, np.float32)
    f = np.asarray(feats, np.float32)
    s = np.asarray(clsbd_feats, np.float32)

    xpad = np.zeros((B, C, H + 4, W), np.float32)
    xpad[:, :, 2:2 + H] = x
    fpad = np.full((B, D, H + 8, W), BIGPAD, np.float32)
    fpad[:, :, 4:4 + H] = f
    spad = np.zeros((B, H + 4, W), np.float32)
    spad[:, 2:2 + H] = s[:, 0]

    in_maps = []
    for i in range(8):
        b, half = i // 2, i % 2
        h0 = half * HP
        in_maps.append({
            "x": np.ascontiguousarray(
                xpad[b, :, h0:h0 + HE].transpose(2, 0, 1)),
            "f": np.ascontiguousarray(
                fpad[b, :, h0:h0 + FE].transpose(2, 0, 1)),
            "s": np.ascontiguousarray(spad[b, h0:h0 + HE].transpose(1, 0)),
        })

    if "nc" not in _cache:
        _cache["nc"] = _build()
    res = run_bass_kernel_spmd(_cache["nc"], in_maps, list(range(8)))
    _last_results = res

    out = np.empty((2, B, C, H, W), np.float32)
    for i in range(8):
        b, half = i // 2, i % 2
        h0 = half * HP
        out[:, b, :, h0:h0 + HP] = res.results[i]["out"].transpose(0, 2, 3, 1)
    return out



# revision 17
# speedup vs baseline: 2.8759x; 2.8759x over previous
"""ClsbdCRF message-passing kernel for 8 Trainium2 NeuronCores.

Sharding: core i handles batch b = i//2 and image-row half i%2 (64 output
rows each, halos sliced host-side).  Per-core SBUF layout puts W=128 on
partitions and (C, H) on the free dimension.

Formulation ("input frame"): for every tap t the reference computes
  msg[c,p] += w_t[p] * xp[c, p+t].
We instead build the tap weight shifted into the *input* frame,
  w'_t[u] = w_t[u - t],
multiply v_t = w'_t * xp (per-tap elementwise product, fp16, DVE 2x mode),
and let the PE do the shift-and-accumulate into PSUM with 0/1 shift-matrix
stationaries:  msg[w] += v_t[w + dy_t]  (dx_t handled as a free-dim offset
baked into the exact 64-wide window of each product tile).

Weight structure exploited:
  * pairwise gaussian symmetry: g1'_t = g1_{-t}; 12 direct tiles cover 12
    taps for free, the other 10 (dy!=0) are small PE partition shifts.
  * clsbd ring-1: g2'_t[u] = s[u] for all 8 taps -> the pos-stream ring-1
    products depend only on dx -> 3 shared products, 8 matmuls.
  * ring-2: g2'_t[u] = max(s[u], s[u+d1], s[u+d2]) with |d|<=1.
  * center tap: msg_neg += c0 * xp via a scaled-identity stationary.
All products/weights fp16 (rel err ~5e-4, tolerance 2e-2); PSUM fp32.
"""

import math
import os

import numpy as np

SKIP = set(os.environ.get("CLSBD_SKIP", "").split(",")) - {""}

B, C, H, W, D = 4, 21, 128, 128, 5
EPS = 1e-5
HP = 64          # output rows per core
HE = HP + 4      # x row extent (halo 2)        frame: q in [0,68), idx = q
FE = HP + 8      # feats row extent (halo 4)    idx = q + 2
SE = HP + 6      # clsbd row extent (halo 3)    idx = q + 1
G1W = 66         # direct-gaussian window: q in [0,66)
BIGPAD = 1000.0
C0N = 2.0 - math.log(EPS)   # center neg weight (final *5 at evac)

RING1 = [(-1, -1), (-1, 0), (-1, 1), (0, -1), (0, 1), (1, -1), (1, 0), (1, 1)]
RING2 = [(-2, -2), (-2, -1), (-2, 0), (-2, 1), (-2, 2), (-1, -2), (-1, 2),
         (0, -2), (0, 2), (1, -2), (1, 2), (2, -2), (2, -1), (2, 0), (2, 1),
         (2, 2)]
EXP1 = [0, 0, 1, 2, 2, 0, 2, 3, 4, 5, 7, 5, 5, 6, 7, 7]
EXP2 = [0, 1, 1, 1, 2, 3, 4, 3, 4, 3, 4, 5, 6, 6, 6, 7]
ALLT = [(dx, dy) for dx in range(-2, 3) for dy in range(-2, 3)
        if (dx, dy) != (0, 0)]
DIRTAPS = [t for t in ALLT if t > (0, 0)]           # direct-gaussian taps
DYS5 = [-2, -1, 0, 1, 2]
JDY = {dy: j for j, dy in enumerate(DYS5)}           # fs / sm slot per dy

# direct-gaussian stack slots, grouped by dy (dx ascending inside a group)
DSTACK = sorted(DIRTAPS, key=lambda t: (t[1], t[0]))
DSLOT = {t: i for i, t in enumerate(DSTACK)}
# mirror psum regions: per direct dy!=0 group (slot range, psum col offset)
MIRGRP = []
_off = 0
for gdy in (1, 2, -1, -2):
    sl = [DSLOT[t] for t in DSTACK if t[1] == gdy]
    MIRGRP.append((gdy, min(sl), len(sl), _off))
    _off += len(sl) * HP

# weight-stack slots (fp16, exact 64-wide windows)
PR1SLOT = {dx: dx + 1 for dx in (-1, 0, 1)}          # 0..2 pos ring-1 (by dx)
PR2SLOT = {t: 3 + k for k, t in enumerate(RING2)}    # 3..18 pos ring-2
NEGSLOT = {t: 19 + i for i, t in enumerate(ALLT)}    # 19..42 neg
NW = 43

FSL = [(0, 512), (512, 1024), (1024, C * HP)]        # psum bank chunks
NMM = {"n": 25 - (1 if "center" in SKIP else 0)
       - (12 if "dir" in SKIP else 0) - (2 if "fs" in SKIP else 0)
       - (10 if "mir" in SKIP else 0), "p": 24}      # matmuls per chunk

_cache = {}


def _build():
    import concourse.bacc as bacc
    import concourse.mybir as mybir
    from concourse.tile import TileContext

    f32 = mybir.dt.float32
    f16 = mybir.dt.float16
    Act = mybir.ActivationFunctionType
    Alu = mybir.AluOpType

    nc = bacc.Bacc()
    x_d = nc.declare_dram_parameter("x", [W, C, HE], f32, isOutput=False)
    f_d = nc.declare_dram_parameter("fs", [W, 5, D, FE], f32, isOutput=False)
    s_d = nc.declare_dram_parameter("ss", [W, 3, SE], f32, isOutput=False)
    m_d = nc.declare_dram_parameter("sm", [W, 6, W], f16, isOutput=False)
    b_d = nc.declare_dram_parameter("bc", [W, 3], f32, isOutput=False)
    o_d = nc.declare_dram_parameter("out", [2, W, C, HP], f32, isOutput=True)

    with TileContext(nc) as tc:
        with (
            tc.tile_pool(name="io", bufs=1) as io,
            tc.tile_pool(name="wk", bufs=1) as wk,
            tc.tile_pool(name="sc", bufs=1) as scp,
            tc.tile_pool(name="vp", bufs=4) as vp,
            tc.tile_pool(name="ps", bufs=1, space="PSUM") as psp,
        ):
            # ---- loads (all plain contiguous DMAs, spread over queues) ----
            x_t = io.tile([W, C, HE], f32, tag="x")
            fs_t = io.tile([W, 5, D, FE], f32, tag="fs")
            ss_t = io.tile([W, 3, SE], f32, tag="ss")
            sm_t = io.tile([W, 6, W], f16, tag="sm")
            bc_t = io.tile([W, 3], f32, tag="bc")
            nc.sync.dma_start(out=x_t[:], in_=x_d[:])
            nc.scalar.dma_start(out=fs_t[:], in_=f_d[:])
            nc.gpsimd.dma_start(out=ss_t[:], in_=s_d[:])
            nc.gpsimd.dma_start(out=sm_t[:], in_=m_d[:])
            nc.gpsimd.dma_start(out=bc_t[:], in_=b_d[:])
            b_eps = bc_t[:, 0:1]
            b_ln2 = bc_t[:, 1:2]
            b_1eps = bc_t[:, 2:3]

            accn = psp.tile([W, 1536], f32, tag="accn")
            accp = psp.tile([W, 1536], f32, tag="accp")
            mir = psp.tile([W, 1024], f32, tag="mir")

            s0 = ss_t[:, 1]                       # s[u] at idx u+1

            # ---- ring-2 clsbd gaussians (gpsimd, needs only ss) ----
            g2r2 = wk.tile([W, 16, HP], f32, tag="g2r2")
            for k, (dx, dy) in enumerate(RING2):
                u0 = 2 + dx
                base = s0[:, u0 + 1:u0 + 65]
                par = sorted({EXP1[k], EXP2[k]})
                d0x = RING1[par[0]][0] - dx
                d0y = RING1[par[0]][1] - dy
                nc.vector.tensor_max(
                    g2r2[:, k], base,
                    ss_t[:, 1 + d0y, u0 + d0x + 1:u0 + d0x + 65])
                if len(par) > 1:
                    d1x = RING1[par[1]][0] - dx
                    d1y = RING1[par[1]][1] - dy
                    nc.vector.tensor_max(
                        g2r2[:, k], g2r2[:, k],
                        ss_t[:, 1 + d1y, u0 + d1x + 1:u0 + d1x + 65])

            # ---- polarness -> xp (fp16) ----
            lnx = scp.tile([W, C, HE], f32, tag="lnx")
            nc.scalar.activation(lnx[:], x_t[:], Act.Ln, bias=b_eps, scale=1.0)
            xl = scp.tile([W, C, HE], f32, tag="xl")
            nc.vector.tensor_mul(xl[:], x_t[:], lnx[:])
            ent = wk.tile([W, HE], f32, tag="ent")
            nc.vector.tensor_reduce(
                out=ent[:], in_=xl[:].rearrange("p c h -> p h c"),
                axis=mybir.AxisListType.X, op=Alu.add)
            pl = wk.tile([W, HE], f32, tag="pl")
            nc.scalar.activation(pl[:], ent[:], Act.Copy,
                                 bias=1.0, scale=1.0 / math.log(C))
            xp16e = io.tile([W, C, HE], f16, tag="xp16e")
            nc.vector.tensor_mul(
                xp16e[:], x_t[:], pl[:, None, :].broadcast_to((W, C, HE)))
            xp16o = io.tile([W, C, G1W], f16, tag="xp16o")
            nc.vector.tensor_copy(xp16o[:], xp16e[:, :, 1:1 + G1W])
            xpc = io.tile([W, C, HP], f16, tag="xpc")
            nc.vector.tensor_copy(xpc[:], xp16e[:, :, 2:2 + HP])

            def xp_win(dx):
                o = 2 + dx
                if o % 2 == 0:
                    return xp16e[:, :, o:o + HP]
                return xp16o[:, :, o - 1:o - 1 + HP]

            # ---- early scalar weights from s only ----
            wstk = wk.tile([W, NW, HP], f16, tag="wstk")
            lnn = wk.tile([W, HE], f32, tag="lnn")
            nc.scalar.activation(lnn[:], s0[:, 1:1 + HE], Act.Ln,
                                 bias=b_eps, scale=1.0)
            for dx in (-1, 0, 1):
                u0 = 2 + dx
                nc.scalar.activation(wstk[:, PR1SLOT[dx]],
                                     s0[:, u0 + 1:u0 + 65], Act.Ln,
                                     bias=b_1eps, scale=-1.0)

            # ---- matmul emission helper (psum accumulation groups) ----
            cnt = {("n", i): 0 for i in range(3)} | {("p", i): 0
                                                     for i in range(3)}

            def emit_mm(stream, vflat, dy):
                ps = accn if stream == "n" else accp
                for ci, (n0, n1) in enumerate(FSL):
                    cnt[(stream, ci)] += 1
                    c = cnt[(stream, ci)]
                    nc.tensor.matmul(ps[:, n0:n1], sm_t[:, JDY[dy]],
                                     vflat[:, n0:n1], start=(c == 1),
                                     stop=(c == NMM[stream]),
                                     skip_group_check=True)

            def product(stream, wslot, dx, dys, eng=None):
                eng = eng or nc.vector
                v = vp.tile([W, C, HP], f16, tag="v")
                eng.tensor_mul(
                    v[:], wstk[:, wslot, None, :].broadcast_to((W, C, HP)),
                    xp_win(dx))
                vflat = v[:].rearrange("p c h -> p (c h)")
                for dy in dys:
                    emit_mm(stream, vflat, dy)

            # center tap first: warms PE early, opens the accn groups
            for ci, (n0, n1) in enumerate(FSL) if "center" not in SKIP else ():
                cnt[("n", ci)] += 1
                nc.tensor.matmul(accn[:, n0:n1], sm_t[:, 5],
                                 xpc[:].rearrange("p c h -> p (c h)")[:, n0:n1],
                                 start=True, stop=(cnt[("n", ci)] == NMM["n"]),
                                 skip_group_check=True)

            # ---- pairwise gaussian chain ----
            dstk = scp.tile([W, 12, D, G1W], f32, tag="dstk")
            for t in DIRTAPS:
                mdx, mdy = t
                nc.vector.tensor_sub(
                    dstk[:, DSLOT[t]], fs_t[:, 2, :, 2:2 + G1W],
                    fs_t[:, JDY[mdy], :, 2 + mdx:2 + mdx + G1W])
            sq = scp.tile([W, 12, D, G1W], f16, tag="sq")
            for a, b2 in ((0, 4), (4, 8), (8, 12)):
                nc.scalar.activation(sq[:, a:b2], dstk[:, a:b2],
                                     Act.Square, bias=0.0, scale=1.0)

            # pos ring-1: 3 shared products (weight depends only on dx),
            # each accumulated with 2-3 different dy shift matrices
            for dx in (-1, 0, 1):
                dys = [dy for dy in (-1, 0, 1) if (dx, dy) != (0, 0)]
                product("p", PR1SLOT[dx], dx, dys)

            # ---- sum over D (fp16 adds, 2x mode) ----
            q01 = scp.tile([W, 12, G1W], f16, tag="q01")
            nc.vector.tensor_add(q01[:], sq[:, :, 0], sq[:, :, 1])
            q23 = scp.tile([W, 12, G1W], f16, tag="q23")
            nc.vector.tensor_add(q23[:], sq[:, :, 2], sq[:, :, 3])
            nc.vector.tensor_add(q01[:], q01[:], q23[:])
            ssum = scp.tile([W, 12, G1W], f16, tag="ssum")
            nc.vector.tensor_add(ssum[:], q01[:], sq[:, :, 4])

            # ---- ring-2 weights (scalar; only need g2r2) + exp ----
            lnn2 = wk.tile([W, 16, HP], f32, tag="lnn2")
            nc.scalar.activation(lnn2[:], g2r2[:], Act.Ln,
                                 bias=b_eps, scale=1.0)
            nc.scalar.activation(wstk[:, 3:19], g2r2[:], Act.Ln,
                                 bias=b_1eps, scale=-1.0)
            g1x2 = wk.tile([W, 12, G1W], f32, tag="g1x2")
            nc.scalar.activation(g1x2[:], ssum[:], Act.Exp,
                                 bias=b_ln2, scale=-0.5)
            for k, (dx, dy) in enumerate(RING2):
                product("p", PR2SLOT[(dx, dy)], dx, [dy])

            def lnn_in1(t):
                if t in RING2:
                    return lnn2[:, RING2.index(t)]
                return lnn[:, 2 + t[0]:2 + t[0] + HP]

            # ---- neg weights: direct + free-shift taps ----
            for t in ALLT:
                dx, dy = t
                if t < (0, 0):
                    src = g1x2[:, DSLOT[(-dx, -dy)], 2 + dx:2 + dx + HP]
                elif dy == 0:                      # (1,0),(2,0): free shift
                    src = g1x2[:, DSLOT[t], 2:2 + HP]
                else:
                    continue
                if ("dir" if t < (0, 0) else "fs") in SKIP:
                    continue
                nc.vector.tensor_sub(wstk[:, NEGSLOT[t]], src, lnn_in1(t))
                product("n", NEGSLOT[t], dx, [dy])

            # ---- mirror taps via PE partition shift ----
            g1den = wk.tile([W, 12, HP], f16, tag="g1den")
            nc.scalar.activation(g1den[:], g1x2[:, :, 2:2 + HP],
                                 Act.Copy, bias=0.0, scale=1.0)
            for gdy, sl0, n, off in MIRGRP:
                # one psum group per bank: later matmuls in bank 6 must not
                # re-assert start (it clears the whole bank)
                bank_first = off % 512 == 0
                bank_last = (off + n * HP) % 512 == 0 or gdy == MIRGRP[-1][0]
                nc.tensor.matmul(
                    mir[:, off:off + n * HP], sm_t[:, JDY[-gdy]],
                    g1den[:, sl0:sl0 + n].rearrange("p s h -> p (s h)"),
                    start=bank_first, stop=bank_last, skip_group_check=True)
            for gdy, sl0, n, off in MIRGRP:
                for gi in range(n):
                    m = DSTACK[sl0 + gi]
                    t = m                 # g1'_m[u] = g1_m[u - m]: shifted
                    if "mir" in SKIP:
                        continue
                    nc.vector.tensor_sub(
                        wstk[:, NEGSLOT[t]],
                        mir[:, off + gi * HP:off + (gi + 1) * HP],
                        lnn_in1(t))
                    product("n", NEGSLOT[t], t[0], [t[1]])

            # ---- evac + stores ----
            on_t = io.tile([W, C, HP], f32, tag="on")
            op_t = io.tile([W, C, HP], f32, tag="op")
            nc.scalar.activation(op_t[:].rearrange("p c h -> p (c h)"),
                                 accp[:, 0:C * HP], Act.Copy,
                                 bias=0.0, scale=-5.0)
            nc.gpsimd.dma_start(out=o_d[1], in_=op_t[:])
            nc.scalar.activation(on_t[:].rearrange("p c h -> p (c h)"),
                                 accn[:, 0:C * HP], Act.Copy,
                                 bias=0.0, scale=5.0)
            nc.sync.dma_start(out=o_d[0], in_=on_t[:])
    nc.finalize()
    return nc


def _host_inputs(input, feats, clsbd_feats):
    x = np.asarray(input, np.float32)
    f = np.asarray(feats, np.float32)
    s = np.asarray(clsbd_feats, np.float32)[:, 0]

    xpad = np.zeros((B, C, H + 4, W), np.float32)
    xpad[:, :, 2:2 + H] = x
    fpad = np.full((B, D, H + 8, W), BIGPAD, np.float32)
    fpad[:, :, 4:4 + H] = f
    spad = np.zeros((B, H + 6, W), np.float32)
    spad[:, 3:3 + H] = s

    def shift_w(arr, dy, fill):
        out = np.full_like(arr, fill)
        if dy >= 0:
            out[..., :W - dy] = arr[..., dy:]
        else:
            out[..., -dy:] = arr[..., :W + dy]
        return out

    sm = np.zeros((W, 6, W), np.float16)
    for j, dy in enumerate(DYS5):
        a, bnd = max(0, -dy), W - max(0, dy)
        for mcol in range(a, bnd):
            sm[mcol + dy, j, mcol] = 1.0
    sm[:, 5][np.arange(W), np.arange(W)] = np.float16(C0N)
    bc = np.tile(np.array([EPS, math.log(2.0), 1.0 + EPS],
                          np.float32), (W, 1))

    maps = []
    for i in range(8):
        b, half = i // 2, i % 2
        h0 = half * HP
        fsw = np.stack([shift_w(fpad[b, :, h0:h0 + FE], dy, BIGPAD)
                        for dy in DYS5])          # [5, D, FE, W]
        ssw = np.stack([shift_w(spad[b, h0:h0 + SE], dy, 0.0)
                        for dy in (-1, 0, 1)])    # [3, SE, W]
        maps.append({
            "x": np.ascontiguousarray(
                xpad[b, :, h0:h0 + HE].transpose(2, 0, 1)),
            "fs": np.ascontiguousarray(fsw.transpose(3, 0, 1, 2)),
            "ss": np.ascontiguousarray(ssw.transpose(2, 0, 1)),
            "sm": sm,
            "bc": bc,
        })
    return maps


_last_results = None


def kernel(input, feats, clsbd_feats, label=None, **_ignored):
    global _last_results
    from concourse.bass_utils import run_bass_kernel_spmd

    in_maps = _host_inputs(input, feats, clsbd_feats)
    if "nc" not in _cache:
        _cache["nc"] = _build()
    res = run_bass_kernel_spmd(_cache["nc"], in_maps, list(range(8)))
    _last_results = res

    out = np.empty((2, B, C, H, W), np.float32)
    for i in range(8):
        b, half = i // 2, i % 2
        h0 = half * HP
        out[:, b, :, h0:h0 + HP] = res.results[i]["out"].transpose(0, 2, 3, 1)
    return out
